# revision 7
# baseline (speedup 1.0000x reference)
"""CTC loss (warp-ctc semantics, size_average=True) on 8 Trainium2 NeuronCores.

Strategy (data-parallel over batch, 4 samples per core):
- Z[t,b] = sum_v exp(acts[t,b,v]) streamed as 32 [128,4000] f32 tiles over
  THREE DMA queues (sync + scalar HW DGE, gpsimd SW DGE); exp + free-dim sum
  fused in one ScalarE activation (accum_out, f32). Host does log Z in f64.
- The alpha recursion runs in the LINEAR domain, bf16, with STATES ON THE
  FREE AXIS (partition shifts are illegal on compute engines; free-dim
  offsets are free). 8 partition rows = 4 samples x {fw, bw}.
- Forward and backward DPs run SIMULTANEOUSLY in the same instructions
  (backward CTC == forward CTC on time-reversed, state-flipped data), meeting
  in the middle: 255 fused steps instead of 511. State = (y_b, ml) where
  y_b = pre-emission blank sums and ml = label alphas; per step exactly 3
  serial DVE ops (the blank emission is a per-row scalar, so it fuses into a
  scalar_tensor_tensor multiply-add):
    STT: y_b' = y_b * pb + shift(ml)
    ADD: y_l  = y_b' + ml
    MUL: ml'  = y_l * pl
  No TensorE, no cross-engine syncs on the serial chain.
- Rescale every RSC steps: accum_out on STT/MUL gives state sums free;
  reciprocal applied via tensor_scalar. Factors folded back in log on host.
- Emission tables (host pre-exp'd: pgl bf16 [8,256*100], pgb f32 [8,256]) and
  init states load over the idle gpsimd queue; result DMAs also go via
  gpsimd so they never block the streaming queues.
- Final: ll_b = log(sum_s y_fw[s]*g_bw[s]) + sum log u - sum log Z (host f64);
  loss = -mean(ll).
"""

import sys
import types

import numpy as np
import ml_dtypes

# ---- shim: provide antenv.axon_hooks (missing in this image) ----------------
_HOOK = [None]
try:
    import antenv.axon_hooks  # noqa: F401
except ImportError:
    try:
        from trn_agent_boot.trn_boot import _ntff_profile_via_ctypes

        _HOOK[0] = _ntff_profile_via_ctypes("/opt/axon/libaxon_pjrt.so")
    except Exception:
        pass
    _m = types.ModuleType("antenv.axon_hooks")
    _m.get_axon_ntff_profile_hook = lambda: _HOOK[0]
    _m.set_axon_ntff_profile_hook = lambda h: _HOOK.__setitem__(0, h)
    sys.modules["antenv.axon_hooks"] = _m
# -----------------------------------------------------------------------------

import concourse.bass as bass
import concourse.mybir as mybir
import concourse.tile as tile
from concourse.bass_utils import run_bass_kernel_spmd
from concourse.vector_clock import ScopedClock


# ---- walrus-compat patches: this walrus rejects Drains with >1 sem wait -----
def _my_drain_and_barrier(self, tick_clock, wait_clock):
    nc = self.nc
    dummy = nc.sync.nop(nofuse=True)
    wait_clock.add_sem_waits(dummy.ins, ScopedClock({None: tick_clock.global_clock}))
    si = dummy.ins.sync_info
    waits = list(si.on_wait) if si is not None else []
    if si is not None and len(waits) > 1:
        dummy.ins.sync_info = mybir.SyncInfo(
            on_wait=[waits[0]], on_update=list(si.on_update)
        )
        for w in waits[1:]:
            n = nc.sync.nop(nofuse=True)
            n.ins.sync_info = mybir.SyncInfo(on_wait=[w], on_update=[])
    nc.sync.drain()
    nc.all_engine_barrier()
    assert self.sems is not None
    popped = nc._tile_sem_poison_stack.pop()
    assert popped is self._sem_poison
    nc.clear_and_free_semaphores(list(self.sems.allocated().values()))
    nc.all_engine_barrier()


def _my_multi_engine_barrier(self, engines):
    for e in engines:
        self.engines[e].drain()
    for inst in self._sem_only_all_engine_barrier_insts(f"aeb{self.next_id()}"):
        self.engines[inst.engine].add_instruction(inst)


tile.TileContext._drain_and_barrier = _my_drain_and_barrier
bass.Bass.multi_engine_barrier = _my_multi_engine_barrier


def _split_multiwait(nc):
    """This walrus build encodes at most one sync-wait per instruction; hoist
    extra waits onto preceding nofuse NOPs on the same engine."""
    n_new = 0
    for fn in nc.m.functions:
        for blk in fn.blocks:
            insts = blk.instructions
            i = 0
            while i < len(insts):
                ins = insts[i]
                si = getattr(ins, "sync_info", None)
                if si is not None and si.on_wait and len(si.on_wait) > 1:
                    waits = list(si.on_wait)
                    ins.sync_info = mybir.SyncInfo(
                        on_wait=[waits[-1]], on_update=list(si.on_update)
                    )
                    new_nops = []
                    for w in waits[:-1]:
                        nop = mybir.InstNoOp(
                            name=f"{ins.name}_wsplit{n_new}",
                            engine=ins.engine,
                            sync_info=mybir.SyncInfo(on_wait=[w], on_update=[]),
                            bass_nofuse=True,
                        )
                        n_new += 1
                        new_nops.append(nop)
                    insts[i:i] = new_nops
                    i += len(new_nops)
                i += 1
    return nc
# -----------------------------------------------------------------------------

T, B, V, L = 512, 32, 8000, 100
NCORES = 8
NB = B // NCORES  # 4 samples per core
KS = 255  # fused fw+bw steps
NK = 256  # table slots
RSC = 32  # rescale every RSC steps
NSITES = len(range(RSC, KS, RSC))  # 7
VH = V // 2  # streaming tile width
NT2 = NB * (T // 128) * 2  # 32 streaming tiles per core
SBUFS = 6  # streaming tile pool depth
F32 = mybir.dt.float32
BF16 = mybir.dt.bfloat16
BFNP = ml_dtypes.bfloat16
ADD = mybir.AluOpType.add
MULT = mybir.AluOpType.mult


def build_program(t_steps=T):
    """Build the per-core Bass program (identical for all cores)."""
    assert t_steps == T
    nc = bass.Bass("TRN2", target_bir_lowering=False, debug=False)

    acts_d = nc.dram_tensor("acts", [NB * T, V], F32, kind="ExternalInput")
    pgl_d = nc.dram_tensor("pgl", [8, NK * L], BF16, kind="ExternalInput")
    pgb_d = nc.dram_tensor("pgb", [8, NK], F32, kind="ExternalInput")
    yb0_d = nc.dram_tensor("yb0", [8, 102], BF16, kind="ExternalInput")
    ml0_d = nc.dram_tensor("ml0", [8, 101], BF16, kind="ExternalInput")

    zacc_d = nc.dram_tensor("zacc", [128, NT2], F32, kind="ExternalOutput")
    ab_d = nc.dram_tensor("ab", [8, 101], BF16, kind="ExternalOutput")
    ml_d = nc.dram_tensor("ml", [8, 101], BF16, kind="ExternalOutput")
    yb2_d = nc.dram_tensor("yb2", [8, 102], BF16, kind="ExternalOutput")
    yl2_d = nc.dram_tensor("yl2", [8, 100], BF16, kind="ExternalOutput")
    ubuf_d = nc.dram_tensor("ubuf", [8, 2 * NSITES], F32, kind="ExternalOutput")

    with tile.TileContext(nc) as tc:
        with (
            tc.tile_pool(name="singles", bufs=1) as singles,
            tc.tile_pool(name="stream", bufs=SBUFS) as stream_pool,
            tc.tile_pool(name="escratch", bufs=1) as escratch_pool,
        ):
            # ---- tables + init state over the idle gpsimd queue -------------
            pgl = singles.tile([8, NK * L], BF16)
            pgb = singles.tile([8, NK], F32)
            YB = singles.tile([8, 102], BF16)
            ML = singles.tile([8, 101], BF16)
            YL = singles.tile([8, 100], BF16)
            AB = singles.tile([8, 101], BF16)
            YB2 = singles.tile([8, 102], BF16)
            YL2 = singles.tile([8, 100], BF16)
            rt = singles.tile([8, 2], F32)
            ub = singles.tile([8, 2 * NSITES], F32)
            nc.gpsimd.dma_start(out=pgl, in_=pgl_d[:, :])
            nc.gpsimd.dma_start(out=pgb, in_=pgb_d[:, :])
            nc.gpsimd.dma_start(out=YB, in_=yb0_d[:, :])
            nc.gpsimd.dma_start(out=ML, in_=ml0_d[:, :])

            site = 0
            for k in range(1, KS + 1):
                at_site = k % RSC == 0 and k < KS
                if at_site:
                    nc.vector.scalar_tensor_tensor(
                        YB[:, 1:102], YB[:, 1:102], pgb[:, k - 1 : k],
                        ML[:, 0:101], op0=MULT, op1=ADD,
                        accum_out=ub[:, 2 * site : 2 * site + 1],
                    )
                else:
                    nc.vector.scalar_tensor_tensor(
                        YB[:, 1:102], YB[:, 1:102], pgb[:, k - 1 : k],
                        ML[:, 0:101], op0=MULT, op1=ADD,
                    )
                nc.vector.tensor_add(YL[:, 0:100], YB[:, 1:101], ML[:, 1:101])
                if at_site:
                    nc.vector.scalar_tensor_tensor(
                        ML[:, 1:101], YL[:, 0:100], 0.0,
                        pgl[:, k * L : k * L + 100], op0=ADD, op1=MULT,
                        accum_out=ub[:, 2 * site + 1 : 2 * site + 2],
                    )
                    nc.vector.tensor_add(
                        rt[:, 1:2],
                        ub[:, 2 * site : 2 * site + 1],
                        ub[:, 2 * site + 1 : 2 * site + 2],
                    )
                    nc.vector.reciprocal(rt[:, 0:1], rt[:, 1:2])
                    nc.vector.tensor_scalar_mul(YB[:, :], YB[:, :], rt[:, 0:1])
                    nc.vector.tensor_scalar_mul(ML[:, :], ML[:, :], rt[:, 0:1])
                    site += 1
                else:
                    nc.vector.tensor_mul(
                        ML[:, 1:101], YL[:, 0:100], pgl[:, k * L : k * L + 100]
                    )
            assert site == NSITES

            # final: alpha_b = y_b * pb_KS; y_256 half-step for the host dot
            nc.vector.tensor_scalar_mul(
                AB[:, 0:101], YB[:, 1:102], pgb[:, KS : KS + 1]
            )
            nc.vector.tensor_add(YB2[:, 1:102], AB[:, 0:101], ML[:, 0:101])
            nc.vector.tensor_add(YL2[:, 0:100], YB2[:, 1:101], ML[:, 1:101])
            nc.gpsimd.dma_start(out=ab_d[:, :], in_=AB)
            nc.gpsimd.dma_start(out=ml_d[:, :], in_=ML)
            nc.gpsimd.dma_start(out=yb2_d[:, :], in_=YB2)
            nc.gpsimd.dma_start(out=yl2_d[:, :], in_=YL2)
            nc.gpsimd.dma_start(out=ubuf_d[:, :], in_=ub)

            # ---- streaming Z = sum_v exp(acts), three DMA queues ------------
            ztile = singles.tile([128, NT2], F32)
            engs = [nc.sync, nc.scalar, nc.gpsimd]

            def tile_ap(it):
                rb, ch = it // 2, it % 2
                return acts_d[rb * 128 : (rb + 1) * 128, ch * VH : (ch + 1) * VH]

            tiles = {}
            for it in range(min(SBUFS, NT2)):
                ta = stream_pool.tile([128, VH], F32, tag="acts")
                engs[it % 3].dma_start(out=ta, in_=tile_ap(it))
                tiles[it] = ta
            for it in range(NT2):
                ta = tiles.pop(it)
                e_t = escratch_pool.tile([128, VH], BF16, tag="escr")
                nc.scalar.activation(
                    e_t, ta, mybir.ActivationFunctionType.Exp,
                    accum_out=ztile[:, it : it + 1],
                )
                nxt = it + SBUFS
                if nxt < NT2:
                    tb = stream_pool.tile([128, VH], F32, tag="acts")
                    engs[nxt % 3].dma_start(out=tb, in_=tile_ap(nxt))
                    tiles[nxt] = tb
            nc.gpsimd.dma_start(out=zacc_d[:, :], in_=ztile)
    _split_multiwait(nc)
    return nc


_NC_CACHE = {}


def _get_program(t_steps=T):
    if t_steps not in _NC_CACHE:
        _NC_CACHE[t_steps] = build_program(t_steps)
    return _NC_CACHE[t_steps]


def make_in_maps(acts, targets, t_steps=T):
    assert t_steps == T
    in_maps = []
    karr = np.arange(NK)
    bidx = np.arange(NB)
    for c in range(NCORES):
        bs = slice(c * NB, (c + 1) * NB)
        acts_c = np.ascontiguousarray(
            acts[:, bs, :].transpose(1, 0, 2).reshape(NB * T, V)
        )
        tg = np.asarray(targets[bs], np.int64)  # [NB, L]
        a = acts[:, bs, :]  # [T, NB, V] f32

        pgb = np.empty((8, NK), np.float32)
        pgb[0:4] = np.exp(a[karr, :, 0]).T
        pgb[4:8] = np.exp(a[T - 1 - karr, :, 0]).T
        lab_fw = a[karr[:, None, None], bidx[None, :, None], tg[None, :, :]]
        lab_bw = a[
            (T - 1 - karr)[:, None, None], bidx[None, :, None], tg[None, :, ::-1]
        ]
        pgl = np.empty((8, NK, L), np.float32)
        pgl[0:4] = np.exp(lab_fw).transpose(1, 0, 2)
        pgl[4:8] = np.exp(lab_bw).transpose(1, 0, 2)
        yb0 = np.zeros((8, 102), np.float32)
        yb0[:, 1] = 1.0
        ml0 = np.zeros((8, 101), np.float32)
        for b in range(NB):
            ml0[b, 1] = np.exp(a[0, b, tg[b, 0]])
            ml0[4 + b, 1] = np.exp(a[T - 1, b, tg[b, L - 1]])
        in_maps.append(
            {
                "acts": acts_c,
                "pgl": np.ascontiguousarray(pgl.reshape(8, NK * L).astype(BFNP)),
                "pgb": pgb,
                "yb0": yb0.astype(BFNP),
                "ml0": ml0.astype(BFNP),
            }
        )
    return in_maps


def finalize(results, t_steps=T):
    """Host-side combine: per-sample log-likelihoods -> scalar loss (f64)."""
    assert t_steps == T
    ntchunk = T // 128
    j101 = np.arange(101)
    j100 = np.arange(100)
    lls = []
    for c in range(NCORES):
        out = results[c]
        zacc = np.asarray(out["zacc"], np.float64)  # [128, NT2]
        AB = np.asarray(out["ab"], np.float64)
        ML = np.asarray(out["ml"], np.float64)
        YB2 = np.asarray(out["yb2"], np.float64)
        YL2 = np.asarray(out["yl2"], np.float64)
        ub = np.asarray(out["ubuf"], np.float64).reshape(8, NSITES, 2)
        logs = np.log(ub.sum(axis=2)).sum(axis=1)  # [8]
        for b in range(NB):
            # z: tile it = rb*2 + ch; rows rb*128..; flat row r = b*T + t
            cols = zacc[:, b * ntchunk * 2 : (b + 1) * ntchunk * 2]
            zb = (cols[:, 0::2] + cols[:, 1::2]).T.reshape(-1)  # [T]
            logz = np.log(zb).sum()
            dot = (YB2[b, 1 + j101] * AB[4 + b, 100 - j101]).sum()
            dot += (YL2[b, j100] * ML[4 + b, 100 - j100]).sum()
            ll = np.log(dot) + logs[b] + logs[4 + b] - logz
            lls.append(ll)
    return -np.sum(lls) / B


def kernel(acts, targets, act_lens, label_lens):
    acts = np.asarray(acts, np.float32)
    targets = np.asarray(targets).astype(np.int64)
    act_lens = np.asarray(act_lens)
    label_lens = np.asarray(label_lens)
    assert acts.shape == (T, B, V), acts.shape
    assert targets.shape == (B, L)
    assert (act_lens == T).all() and (label_lens == L).all(), "only full lens supported"
    assert (targets[:, 1:] != targets[:, :-1]).all(), "adjacent repeats unsupported"

    nc = _get_program(T)
    in_maps = make_in_maps(acts, targets, T)
    res = run_bass_kernel_spmd(nc, in_maps, core_ids=list(range(NCORES)))
    return np.float32(finalize(res.results, T))


if __name__ == "__main__":
    rng = np.random.default_rng(0)
    acts = rng.standard_normal((T, B, V)).astype(np.float32)
    targets = rng.integers(1, V, (B, L)).astype(np.int32)
    for bb in range(B):
        while (targets[bb, 1:] == targets[bb, :-1]).any():
            targets[bb] = rng.integers(1, V, (L,)).astype(np.int32)
    act_lens = np.full(B, T, np.int32)
    label_lens = np.full(B, L, np.int32)
    out = kernel(acts, targets, act_lens, label_lens)
    print("kernel loss:", out)
    from ctc_numpy import ctc_ref_numpy

    ref = ctc_ref_numpy(acts, targets, act_lens, label_lens)
    print("ref    loss:", ref, " rel err:", abs(out - ref) / abs(ref))


# revision 10
# speedup vs baseline: 1.0613x; 1.0613x over previous
"""CTC loss (warp-ctc semantics, size_average=True) on 8 Trainium2 NeuronCores.

Strategy (data-parallel over batch, 4 samples per core):
- Z[t,b] = sum_v exp(acts[t,b,v]) streamed as 32 [128,4000] f32 tiles over
  THREE DMA queues (sync + scalar HW DGE, gpsimd SW DGE); exp + free-dim sum
  fused in one ScalarE activation (accum_out, f32). Host does log Z in f64.
- The alpha recursion runs in the LINEAR domain, bf16, with STATES ON THE
  FREE AXIS (partition shifts are illegal on compute engines; free-dim
  offsets are free). 8 partition rows = 4 samples x {fw, bw}.
- Forward and backward DPs run SIMULTANEOUSLY in the same instructions
  (backward CTC == forward CTC on time-reversed, state-flipped data), meeting
  in the middle: 255 fused steps instead of 511. State = (y_b, ml) where
  y_b = pre-emission blank sums and ml = label alphas; per step exactly 3
  serial DVE ops (the blank emission is a per-row scalar, so it fuses into a
  scalar_tensor_tensor multiply-add):
    STT: y_b' = y_b * pb + shift(ml)
    ADD: y_l  = y_b' + ml
    MUL: ml'  = y_l * pl
  No TensorE, no cross-engine syncs on the serial chain.
- Rescale every RSC steps: accum_out on STT/MUL gives state sums free;
  reciprocal applied via tensor_scalar. Factors folded back in log on host.
- Emission tables (host pre-exp'd: pgl bf16 [8,256*100], pgb f32 [8,256]) and
  init states load over the idle gpsimd queue; result DMAs also go via
  gpsimd so they never block the streaming queues.
- Final: ll_b = log(sum_s y_fw[s]*g_bw[s]) + sum log u - sum log Z (host f64);
  loss = -mean(ll).
"""

import sys
import types

import numpy as np
import ml_dtypes

# ---- shim: provide antenv.axon_hooks (missing in this image) ----------------
_HOOK = [None]
try:
    import antenv.axon_hooks  # noqa: F401
except ImportError:
    try:
        from trn_agent_boot.trn_boot import _ntff_profile_via_ctypes

        _HOOK[0] = _ntff_profile_via_ctypes("/opt/axon/libaxon_pjrt.so")
    except Exception:
        pass
    _m = types.ModuleType("antenv.axon_hooks")
    _m.get_axon_ntff_profile_hook = lambda: _HOOK[0]
    _m.set_axon_ntff_profile_hook = lambda h: _HOOK.__setitem__(0, h)
    sys.modules["antenv.axon_hooks"] = _m
# -----------------------------------------------------------------------------

import concourse.bass as bass
import concourse.mybir as mybir
import concourse.tile as tile
from concourse.bass_utils import run_bass_kernel_spmd
from concourse.vector_clock import ScopedClock


# ---- walrus-compat patches: this walrus rejects Drains with >1 sem wait -----
def _my_drain_and_barrier(self, tick_clock, wait_clock):
    nc = self.nc
    dummy = nc.sync.nop(nofuse=True)
    wait_clock.add_sem_waits(dummy.ins, ScopedClock({None: tick_clock.global_clock}))
    si = dummy.ins.sync_info
    waits = list(si.on_wait) if si is not None else []
    if si is not None and len(waits) > 1:
        dummy.ins.sync_info = mybir.SyncInfo(
            on_wait=[waits[0]], on_update=list(si.on_update)
        )
        for w in waits[1:]:
            n = nc.sync.nop(nofuse=True)
            n.ins.sync_info = mybir.SyncInfo(on_wait=[w], on_update=[])
    nc.sync.drain()
    nc.all_engine_barrier()
    assert self.sems is not None
    popped = nc._tile_sem_poison_stack.pop()
    assert popped is self._sem_poison
    nc.clear_and_free_semaphores(list(self.sems.allocated().values()))
    nc.all_engine_barrier()


def _my_multi_engine_barrier(self, engines):
    for e in engines:
        self.engines[e].drain()
    for inst in self._sem_only_all_engine_barrier_insts(f"aeb{self.next_id()}"):
        self.engines[inst.engine].add_instruction(inst)


tile.TileContext._drain_and_barrier = _my_drain_and_barrier
bass.Bass.multi_engine_barrier = _my_multi_engine_barrier


def _split_multiwait(nc):
    """This walrus build encodes at most one sync-wait per instruction; hoist
    extra waits onto preceding nofuse NOPs on the same engine."""
    n_new = 0
    for fn in nc.m.functions:
        for blk in fn.blocks:
            insts = blk.instructions
            i = 0
            while i < len(insts):
                ins = insts[i]
                si = getattr(ins, "sync_info", None)
                if si is not None and si.on_wait and len(si.on_wait) > 1:
                    waits = list(si.on_wait)
                    ins.sync_info = mybir.SyncInfo(
                        on_wait=[waits[-1]], on_update=list(si.on_update)
                    )
                    new_nops = []
                    for w in waits[:-1]:
                        nop = mybir.InstNoOp(
                            name=f"{ins.name}_wsplit{n_new}",
                            engine=ins.engine,
                            sync_info=mybir.SyncInfo(on_wait=[w], on_update=[]),
                            bass_nofuse=True,
                        )
                        n_new += 1
                        new_nops.append(nop)
                    insts[i:i] = new_nops
                    i += len(new_nops)
                i += 1
    return nc
# -----------------------------------------------------------------------------

T, B, V, L = 512, 32, 8000, 100
NCORES = 8
NB = B // NCORES  # 4 samples per core
KS = 255  # fused fw+bw steps
NK = 256  # table slots
RSC = 32  # rescale every RSC steps
NSITES = len(range(RSC, KS, RSC))  # 7
NT2 = NB * (T // 128)  # 16 streaming tiles per core (32KB descriptors)
SBUFS = 3  # streaming tile pool depth
F32 = mybir.dt.float32
BF16 = mybir.dt.bfloat16
BFNP = ml_dtypes.bfloat16
ADD = mybir.AluOpType.add
MULT = mybir.AluOpType.mult


def build_program(t_steps=T):
    """Build the per-core Bass program (identical for all cores)."""
    assert t_steps == T
    nc = bass.Bass("TRN2", target_bir_lowering=False, debug=False)

    acts_d = nc.dram_tensor("acts", [NB * T, V], F32, kind="ExternalInput")
    pgl_d = nc.dram_tensor("pgl", [8, NK * L], BF16, kind="ExternalInput")
    pgb_d = nc.dram_tensor("pgb", [8, NK], F32, kind="ExternalInput")
    yb0_d = nc.dram_tensor("yb0", [8, 102], BF16, kind="ExternalInput")
    ml0_d = nc.dram_tensor("ml0", [8, 101], BF16, kind="ExternalInput")

    zacc_d = nc.dram_tensor("zacc", [128, NT2], F32, kind="ExternalOutput")
    ab_d = nc.dram_tensor("ab", [8, 101], BF16, kind="ExternalOutput")
    ml_d = nc.dram_tensor("ml", [8, 101], BF16, kind="ExternalOutput")
    yb2_d = nc.dram_tensor("yb2", [8, 102], BF16, kind="ExternalOutput")
    yl2_d = nc.dram_tensor("yl2", [8, 100], BF16, kind="ExternalOutput")
    ubuf_d = nc.dram_tensor("ubuf", [8, 2 * NSITES], F32, kind="ExternalOutput")

    with tile.TileContext(nc) as tc:
        with (
            tc.tile_pool(name="singles", bufs=1) as singles,
            tc.tile_pool(name="stream", bufs=SBUFS) as stream_pool,
            tc.tile_pool(name="escratch", bufs=1) as escratch_pool,
        ):
            # ---- tables + init state over the idle gpsimd queue -------------
            pgl = singles.tile([8, NK * L], BF16)
            pgb = singles.tile([8, NK], F32)
            YB = singles.tile([8, 102], BF16)
            ML = singles.tile([8, 101], BF16)
            YL = singles.tile([8, 100], BF16)
            AB = singles.tile([8, 101], BF16)
            YB2 = singles.tile([8, 102], BF16)
            YL2 = singles.tile([8, 100], BF16)
            rt = singles.tile([8, 2], F32)
            ub = singles.tile([8, 2 * NSITES], F32)
            nc.gpsimd.dma_start(out=pgl, in_=pgl_d[:, :])
            nc.gpsimd.dma_start(out=pgb, in_=pgb_d[:, :])
            nc.gpsimd.dma_start(out=YB, in_=yb0_d[:, :])
            nc.gpsimd.dma_start(out=ML, in_=ml0_d[:, :])

            site = 0
            for k in range(1, KS + 1):
                at_site = k % RSC == 0 and k < KS
                if at_site:
                    nc.vector.scalar_tensor_tensor(
                        YB[:, 1:102], YB[:, 1:102], pgb[:, k - 1 : k],
                        ML[:, 0:101], op0=MULT, op1=ADD,
                        accum_out=ub[:, 2 * site : 2 * site + 1],
                    )
                else:
                    nc.vector.scalar_tensor_tensor(
                        YB[:, 1:102], YB[:, 1:102], pgb[:, k - 1 : k],
                        ML[:, 0:101], op0=MULT, op1=ADD,
                    )
                nc.vector.tensor_add(YL[:, 0:100], YB[:, 1:101], ML[:, 1:101])
                if at_site:
                    nc.vector.scalar_tensor_tensor(
                        ML[:, 1:101], YL[:, 0:100], 0.0,
                        pgl[:, k * L : k * L + 100], op0=ADD, op1=MULT,
                        accum_out=ub[:, 2 * site + 1 : 2 * site + 2],
                    )
                    nc.vector.tensor_add(
                        rt[:, 1:2],
                        ub[:, 2 * site : 2 * site + 1],
                        ub[:, 2 * site + 1 : 2 * site + 2],
                    )
                    nc.vector.reciprocal(rt[:, 0:1], rt[:, 1:2])
                    nc.vector.tensor_scalar_mul(YB[:, :], YB[:, :], rt[:, 0:1])
                    nc.vector.tensor_scalar_mul(ML[:, :], ML[:, :], rt[:, 0:1])
                    site += 1
                else:
                    nc.vector.tensor_mul(
                        ML[:, 1:101], YL[:, 0:100], pgl[:, k * L : k * L + 100]
                    )
            assert site == NSITES

            # final: alpha_b = y_b * pb_KS; y_256 half-step for the host dot
            nc.vector.tensor_scalar_mul(
                AB[:, 0:101], YB[:, 1:102], pgb[:, KS : KS + 1]
            )
            nc.vector.tensor_add(YB2[:, 1:102], AB[:, 0:101], ML[:, 0:101])
            nc.vector.tensor_add(YL2[:, 0:100], YB2[:, 1:101], ML[:, 1:101])

            # ---- streaming Z = sum_v exp(acts), two HW DMA queues -----------
            ztile = singles.tile([128, NT2], F32)
            engs = [nc.sync, nc.scalar]

            def tile_ap(it):
                return acts_d[it * 128 : (it + 1) * 128, :]

            tiles = {}
            for it in range(min(SBUFS, NT2)):
                ta = stream_pool.tile([128, V], F32, tag="acts")
                engs[it % 2].dma_start(out=ta, in_=tile_ap(it))
                tiles[it] = ta
            for it in range(NT2):
                ta = tiles.pop(it)
                e_t = escratch_pool.tile([128, V], BF16, tag="escr")
                nc.scalar.activation(
                    e_t, ta, mybir.ActivationFunctionType.Exp,
                    accum_out=ztile[:, it : it + 1],
                )
                nxt = it + SBUFS
                if nxt < NT2:
                    tb = stream_pool.tile([128, V], F32, tag="acts")
                    engs[nxt % 2].dma_start(out=tb, in_=tile_ap(nxt))
                    tiles[nxt] = tb

            # result DMAs on the idle gpsimd queue, emitted last so they can
            # never sit ahead of streaming work in any queue
            nc.gpsimd.dma_start(out=ab_d[:, :], in_=AB)
            nc.gpsimd.dma_start(out=ml_d[:, :], in_=ML)
            nc.gpsimd.dma_start(out=yb2_d[:, :], in_=YB2)
            nc.gpsimd.dma_start(out=yl2_d[:, :], in_=YL2)
            nc.gpsimd.dma_start(out=ubuf_d[:, :], in_=ub)
            nc.gpsimd.dma_start(out=zacc_d[:, :], in_=ztile)
    _split_multiwait(nc)
    return nc


_NC_CACHE = {}


def _get_program(t_steps=T):
    if t_steps not in _NC_CACHE:
        _NC_CACHE[t_steps] = build_program(t_steps)
    return _NC_CACHE[t_steps]


def make_in_maps(acts, targets, t_steps=T):
    assert t_steps == T
    in_maps = []
    karr = np.arange(NK)
    bidx = np.arange(NB)
    for c in range(NCORES):
        bs = slice(c * NB, (c + 1) * NB)
        acts_c = np.ascontiguousarray(
            acts[:, bs, :].transpose(1, 0, 2).reshape(NB * T, V)
        )
        tg = np.asarray(targets[bs], np.int64)  # [NB, L]
        a = acts[:, bs, :]  # [T, NB, V] f32

        pgb = np.empty((8, NK), np.float32)
        pgb[0:4] = np.exp(a[karr, :, 0]).T
        pgb[4:8] = np.exp(a[T - 1 - karr, :, 0]).T
        lab_fw = a[karr[:, None, None], bidx[None, :, None], tg[None, :, :]]
        lab_bw = a[
            (T - 1 - karr)[:, None, None], bidx[None, :, None], tg[None, :, ::-1]
        ]
        pgl = np.empty((8, NK, L), np.float32)
        pgl[0:4] = np.exp(lab_fw).transpose(1, 0, 2)
        pgl[4:8] = np.exp(lab_bw).transpose(1, 0, 2)
        yb0 = np.zeros((8, 102), np.float32)
        yb0[:, 1] = 1.0
        ml0 = np.zeros((8, 101), np.float32)
        for b in range(NB):
            ml0[b, 1] = np.exp(a[0, b, tg[b, 0]])
            ml0[4 + b, 1] = np.exp(a[T - 1, b, tg[b, L - 1]])
        in_maps.append(
            {
                "acts": acts_c,
                "pgl": np.ascontiguousarray(pgl.reshape(8, NK * L).astype(BFNP)),
                "pgb": pgb,
                "yb0": yb0.astype(BFNP),
                "ml0": ml0.astype(BFNP),
            }
        )
    return in_maps


def finalize(results, t_steps=T):
    """Host-side combine: per-sample log-likelihoods -> scalar loss (f64)."""
    assert t_steps == T
    ntchunk = T // 128
    j101 = np.arange(101)
    j100 = np.arange(100)
    lls = []
    for c in range(NCORES):
        out = results[c]
        zacc = np.asarray(out["zacc"], np.float64)  # [128, NT2]
        AB = np.asarray(out["ab"], np.float64)
        ML = np.asarray(out["ml"], np.float64)
        YB2 = np.asarray(out["yb2"], np.float64)
        YL2 = np.asarray(out["yl2"], np.float64)
        ub = np.asarray(out["ubuf"], np.float64).reshape(8, NSITES, 2)
        logs = np.log(ub.sum(axis=2)).sum(axis=1)  # [8]
        for b in range(NB):
            # z col it covers flat rows it*128..; flat row r = b*T + t
            zb = zacc[:, b * ntchunk : (b + 1) * ntchunk].T.reshape(-1)  # [T]
            logz = np.log(zb).sum()
            dot = (YB2[b, 1 + j101] * AB[4 + b, 100 - j101]).sum()
            dot += (YL2[b, j100] * ML[4 + b, 100 - j100]).sum()
            ll = np.log(dot) + logs[b] + logs[4 + b] - logz
            lls.append(ll)
    return -np.sum(lls) / B


def kernel(acts, targets, act_lens, label_lens):
    acts = np.asarray(acts, np.float32)
    targets = np.asarray(targets).astype(np.int64)
    act_lens = np.asarray(act_lens)
    label_lens = np.asarray(label_lens)
    assert acts.shape == (T, B, V), acts.shape
    assert targets.shape == (B, L)
    assert (act_lens == T).all() and (label_lens == L).all(), "only full lens supported"
    assert (targets[:, 1:] != targets[:, :-1]).all(), "adjacent repeats unsupported"

    nc = _get_program(T)
    in_maps = make_in_maps(acts, targets, T)
    res = run_bass_kernel_spmd(nc, in_maps, core_ids=list(range(NCORES)))
    return np.float32(finalize(res.results, T))


if __name__ == "__main__":
    rng = np.random.default_rng(0)
    acts = rng.standard_normal((T, B, V)).astype(np.float32)
    targets = rng.integers(1, V, (B, L)).astype(np.int32)
    for bb in range(B):
        while (targets[bb, 1:] == targets[bb, :-1]).any():
            targets[bb] = rng.integers(1, V, (L,)).astype(np.int32)
    act_lens = np.full(B, T, np.int32)
    label_lens = np.full(B, L, np.int32)
    out = kernel(acts, targets, act_lens, label_lens)
    print("kernel loss:", out)
    from ctc_numpy import ctc_ref_numpy

    ref = ctc_ref_numpy(acts, targets, act_lens, label_lens)
    print("ref    loss:", ref, " rel err:", abs(out - ref) / abs(ref))


# revision 15
# speedup vs baseline: 1.0663x; 1.0047x over previous
"""CTC loss (warp-ctc semantics, size_average=True) on 8 Trainium2 NeuronCores.

Strategy (data-parallel over batch, 4 samples per core):
- Z[t,b] = sum_v exp(acts[t,b,v]) streamed as 32 [128,4000] f32 tiles over
  THREE DMA queues (sync + scalar HW DGE, gpsimd SW DGE); exp + free-dim sum
  fused in one ScalarE activation (accum_out, f32). Host does log Z in f64.
- The alpha recursion runs in the LINEAR domain, bf16, with STATES ON THE
  FREE AXIS (partition shifts are illegal on compute engines; free-dim
  offsets are free). 8 partition rows = 4 samples x {fw, bw}.
- Forward and backward DPs run SIMULTANEOUSLY in the same instructions
  (backward CTC == forward CTC on time-reversed, state-flipped data), meeting
  in the middle: 255 fused steps instead of 511. State = (y_b, ml) where
  y_b = pre-emission blank sums and ml = label alphas; per step exactly 3
  serial DVE ops (the blank emission is a per-row scalar, so it fuses into a
  scalar_tensor_tensor multiply-add):
    STT: y_b' = y_b * pb + shift(ml)
    ADD: y_l  = y_b' + ml
    MUL: ml'  = y_l * pl
  No TensorE, no cross-engine syncs on the serial chain.
- Rescale every RSC steps: accum_out on STT/MUL gives state sums free;
  reciprocal applied via tensor_scalar. Factors folded back in log on host.
- Emission tables (host pre-exp'd: pgl bf16 [8,256*100], pgb f32 [8,256]) and
  init states load over the idle gpsimd queue; result DMAs also go via
  gpsimd so they never block the streaming queues.
- Final: ll_b = log(sum_s y_fw[s]*g_bw[s]) + sum log u - sum log Z (host f64);
  loss = -mean(ll).
"""

import sys
import types

import numpy as np
import ml_dtypes

# ---- shim: provide antenv.axon_hooks (missing in this image) ----------------
_HOOK = [None]
try:
    import antenv.axon_hooks  # noqa: F401
except ImportError:
    try:
        from trn_agent_boot.trn_boot import _ntff_profile_via_ctypes

        _HOOK[0] = _ntff_profile_via_ctypes("/opt/axon/libaxon_pjrt.so")
    except Exception:
        pass
    _m = types.ModuleType("antenv.axon_hooks")
    _m.get_axon_ntff_profile_hook = lambda: _HOOK[0]
    _m.set_axon_ntff_profile_hook = lambda h: _HOOK.__setitem__(0, h)
    sys.modules["antenv.axon_hooks"] = _m
# -----------------------------------------------------------------------------

import concourse.bass as bass
import concourse.mybir as mybir
import concourse.tile as tile
from concourse.bass_utils import run_bass_kernel_spmd
from concourse.vector_clock import ScopedClock


# ---- walrus-compat patches: this walrus rejects Drains with >1 sem wait -----
def _my_drain_and_barrier(self, tick_clock, wait_clock):
    nc = self.nc
    dummy = nc.sync.nop(nofuse=True)
    wait_clock.add_sem_waits(dummy.ins, ScopedClock({None: tick_clock.global_clock}))
    si = dummy.ins.sync_info
    waits = list(si.on_wait) if si is not None else []
    if si is not None and len(waits) > 1:
        dummy.ins.sync_info = mybir.SyncInfo(
            on_wait=[waits[0]], on_update=list(si.on_update)
        )
        for w in waits[1:]:
            n = nc.sync.nop(nofuse=True)
            n.ins.sync_info = mybir.SyncInfo(on_wait=[w], on_update=[])
    nc.sync.drain()
    nc.all_engine_barrier()
    assert self.sems is not None
    popped = nc._tile_sem_poison_stack.pop()
    assert popped is self._sem_poison
    nc.clear_and_free_semaphores(list(self.sems.allocated().values()))
    nc.all_engine_barrier()


def _my_multi_engine_barrier(self, engines):
    for e in engines:
        self.engines[e].drain()
    for inst in self._sem_only_all_engine_barrier_insts(f"aeb{self.next_id()}"):
        self.engines[inst.engine].add_instruction(inst)


tile.TileContext._drain_and_barrier = _my_drain_and_barrier
bass.Bass.multi_engine_barrier = _my_multi_engine_barrier


def _split_multiwait(nc):
    """This walrus build encodes at most one sync-wait per instruction; hoist
    extra waits onto preceding nofuse NOPs on the same engine."""
    n_new = 0
    for fn in nc.m.functions:
        for blk in fn.blocks:
            insts = blk.instructions
            i = 0
            while i < len(insts):
                ins = insts[i]
                si = getattr(ins, "sync_info", None)
                if si is not None and si.on_wait and len(si.on_wait) > 1:
                    waits = list(si.on_wait)
                    ins.sync_info = mybir.SyncInfo(
                        on_wait=[waits[-1]], on_update=list(si.on_update)
                    )
                    new_nops = []
                    for w in waits[:-1]:
                        nop = mybir.InstNoOp(
                            name=f"{ins.name}_wsplit{n_new}",
                            engine=ins.engine,
                            sync_info=mybir.SyncInfo(on_wait=[w], on_update=[]),
                            bass_nofuse=True,
                        )
                        n_new += 1
                        new_nops.append(nop)
                    insts[i:i] = new_nops
                    i += len(new_nops)
                i += 1
    return nc
# -----------------------------------------------------------------------------

T, B, V, L = 512, 32, 8000, 100
NCORES = 8
NB = B // NCORES  # 4 samples per core
KS = 255  # fused fw+bw steps
NK = 256  # table slots
RSC = 32  # rescale every RSC steps
NSITES = len(range(RSC, KS, RSC))  # 7
NT2 = NB * (T // 128)  # 16 streaming tiles per core (32KB descriptors)
SBUFS = 3  # streaming tile pool depth
F32 = mybir.dt.float32
BF16 = mybir.dt.bfloat16
BFNP = ml_dtypes.bfloat16
ADD = mybir.AluOpType.add
MULT = mybir.AluOpType.mult


def build_program(t_steps=T):
    """Build the per-core Bass program (identical for all cores)."""
    assert t_steps == T
    nc = bass.Bass("TRN2", target_bir_lowering=False, debug=False)

    acts_d = nc.dram_tensor("acts", [NB * T, V], F32, kind="ExternalInput")
    pgl_d = nc.dram_tensor("pgl", [8, NK * L], BF16, kind="ExternalInput")
    pgb_d = nc.dram_tensor("pgb", [8, NK], F32, kind="ExternalInput")
    ab0_d = nc.dram_tensor("ab0", [8, 102], BF16, kind="ExternalInput")
    ml0_d = nc.dram_tensor("ml0", [8, 101], BF16, kind="ExternalInput")

    zacc_d = nc.dram_tensor("zacc", [128, NT2], F32, kind="ExternalOutput")
    ab_d = nc.dram_tensor("ab", [8, 102], BF16, kind="ExternalOutput")
    ml_d = nc.dram_tensor("ml", [8, 101], BF16, kind="ExternalOutput")
    yb2_d = nc.dram_tensor("yb2", [8, 102], BF16, kind="ExternalOutput")
    yl2_d = nc.dram_tensor("yl2", [8, 100], BF16, kind="ExternalOutput")
    ubuf_d = nc.dram_tensor("ubuf", [8, 2 * NSITES], F32, kind="ExternalOutput")

    with tile.TileContext(nc) as tc:
        with (
            tc.tile_pool(name="singles", bufs=1) as singles,
            tc.tile_pool(name="stream", bufs=SBUFS) as stream_pool,
            tc.tile_pool(name="escratch", bufs=1) as escratch_pool,
        ):
            # ---- tables + init state over the sync queue (fast, first) ------
            pgl = singles.tile([8, NK * L], BF16)
            pgb = singles.tile([8, NK], F32)
            AB = singles.tile([8, 102], BF16)  # alpha_blank, f=1..101
            ML = singles.tile([8, 101], BF16)  # f0 guard, alpha_label f=1..100
            YB = singles.tile([8, 102], BF16)  # y_blank
            YL = singles.tile([8, 100], BF16)  # y_label
            rt = singles.tile([8, 2], F32)
            ub = singles.tile([8, 2 * NSITES], F32)
            nc.sync.dma_start(out=pgb, in_=pgb_d[:, :])
            nc.sync.dma_start(out=AB, in_=ab0_d[:, :])
            nc.sync.dma_start(out=ML, in_=ml0_d[:, :])
            nc.sync.dma_start(out=pgl, in_=pgl_d[:, :])

            site = 0
            for k in range(1, KS + 1):
                at_site = k % RSC == 0 and k < KS
                # ADD1: y_b[j] = a_b[j] + a_l[j-1]
                nc.vector.tensor_add(YB[:, 1:102], AB[:, 1:102], ML[:, 0:101])
                # ADD2: y_l[j] = y_b[j] + a_l[j]
                nc.vector.tensor_add(YL[:, 0:100], YB[:, 1:101], ML[:, 1:101])
                if at_site:
                    nc.vector.tensor_scalar(
                        AB[:, 1:102], YB[:, 1:102], pgb[:, k : k + 1], 0.0,
                        op0=MULT, op1=ADD,
                        accum_out=ub[:, 2 * site : 2 * site + 1],
                    )
                    nc.vector.scalar_tensor_tensor(
                        ML[:, 1:101], YL[:, 0:100], 0.0,
                        pgl[:, k * L : k * L + 100], op0=ADD, op1=MULT,
                        accum_out=ub[:, 2 * site + 1 : 2 * site + 2],
                    )
                    nc.vector.tensor_add(
                        rt[:, 1:2],
                        ub[:, 2 * site : 2 * site + 1],
                        ub[:, 2 * site + 1 : 2 * site + 2],
                    )
                    nc.vector.reciprocal(rt[:, 0:1], rt[:, 1:2])
                    nc.vector.tensor_scalar_mul(AB[:, :], AB[:, :], rt[:, 0:1])
                    nc.vector.tensor_scalar_mul(ML[:, :], ML[:, :], rt[:, 0:1])
                    site += 1
                else:
                    nc.vector.tensor_scalar_mul(
                        AB[:, 1:102], YB[:, 1:102], pgb[:, k : k + 1]
                    )
                    nc.vector.tensor_mul(
                        ML[:, 1:101], YL[:, 0:100], pgl[:, k * L : k * L + 100]
                    )
            assert site == NSITES

            # final half-step: y_256 for the host dot product
            nc.vector.tensor_add(YB[:, 1:102], AB[:, 1:102], ML[:, 0:101])
            nc.vector.tensor_add(YL[:, 0:100], YB[:, 1:101], ML[:, 1:101])

            # ---- streaming Z = sum_v exp(acts), two HW DMA queues -----------
            ztile = singles.tile([128, NT2], F32)
            engs = [nc.sync, nc.scalar]

            def tile_ap(it):
                return acts_d[it * 128 : (it + 1) * 128, :]

            tiles = {}
            for it in range(min(SBUFS, NT2)):
                ta = stream_pool.tile([128, V], F32, tag="acts")
                engs[it % 2].dma_start(out=ta, in_=tile_ap(it))
                tiles[it] = ta
            for it in range(NT2):
                ta = tiles.pop(it)
                e_t = escratch_pool.tile([128, V], BF16, tag="escr")
                nc.scalar.activation(
                    e_t, ta, mybir.ActivationFunctionType.Exp,
                    accum_out=ztile[:, it : it + 1],
                )
                nxt = it + SBUFS
                if nxt < NT2:
                    tb = stream_pool.tile([128, V], F32, tag="acts")
                    engs[nxt % 2].dma_start(out=tb, in_=tile_ap(nxt))
                    tiles[nxt] = tb

            # result DMAs emitted last so they can never sit ahead of
            # streaming work in any queue
            nc.sync.dma_start(out=ab_d[:, :], in_=AB)
            nc.sync.dma_start(out=ml_d[:, :], in_=ML)
            nc.sync.dma_start(out=yb2_d[:, :], in_=YB)
            nc.sync.dma_start(out=yl2_d[:, :], in_=YL)
            nc.gpsimd.dma_start(out=ubuf_d[:, :], in_=ub)
            nc.gpsimd.dma_start(out=zacc_d[:, :], in_=ztile)
    _split_multiwait(nc)
    return nc


_NC_CACHE = {}


def _get_program(t_steps=T):
    if t_steps not in _NC_CACHE:
        _NC_CACHE[t_steps] = build_program(t_steps)
    return _NC_CACHE[t_steps]


def make_in_maps(acts, targets, t_steps=T):
    assert t_steps == T
    in_maps = []
    karr = np.arange(NK)
    bidx = np.arange(NB)
    for c in range(NCORES):
        bs = slice(c * NB, (c + 1) * NB)
        acts_c = np.ascontiguousarray(
            acts[:, bs, :].transpose(1, 0, 2).reshape(NB * T, V)
        )
        tg = np.asarray(targets[bs], np.int64)  # [NB, L]
        a = acts[:, bs, :]  # [T, NB, V] f32

        pgb = np.empty((8, NK), np.float32)
        pgb[0:4] = np.exp(a[karr, :, 0]).T
        pgb[4:8] = np.exp(a[T - 1 - karr, :, 0]).T
        lab_fw = a[karr[:, None, None], bidx[None, :, None], tg[None, :, :]]
        lab_bw = a[
            (T - 1 - karr)[:, None, None], bidx[None, :, None], tg[None, :, ::-1]
        ]
        pgl = np.empty((8, NK, L), np.float32)
        pgl[0:4] = np.exp(lab_fw).transpose(1, 0, 2)
        pgl[4:8] = np.exp(lab_bw).transpose(1, 0, 2)
        ab0 = np.zeros((8, 102), np.float32)
        ml0 = np.zeros((8, 101), np.float32)
        for b in range(NB):
            ab0[b, 1] = np.exp(a[0, b, 0])
            ab0[4 + b, 1] = np.exp(a[T - 1, b, 0])
            ml0[b, 1] = np.exp(a[0, b, tg[b, 0]])
            ml0[4 + b, 1] = np.exp(a[T - 1, b, tg[b, L - 1]])
        in_maps.append(
            {
                "acts": acts_c,
                "pgl": np.ascontiguousarray(pgl.reshape(8, NK * L).astype(BFNP)),
                "pgb": pgb,
                "ab0": ab0.astype(BFNP),
                "ml0": ml0.astype(BFNP),
            }
        )
    return in_maps


def finalize(results, t_steps=T):
    """Host-side combine: per-sample log-likelihoods -> scalar loss (f64)."""
    assert t_steps == T
    ntchunk = T // 128
    j101 = np.arange(101)
    j100 = np.arange(100)
    lls = []
    for c in range(NCORES):
        out = results[c]
        zacc = np.asarray(out["zacc"], np.float64)  # [128, NT2]
        AB = np.asarray(out["ab"], np.float64)
        ML = np.asarray(out["ml"], np.float64)
        YB2 = np.asarray(out["yb2"], np.float64)
        YL2 = np.asarray(out["yl2"], np.float64)
        ub = np.asarray(out["ubuf"], np.float64).reshape(8, NSITES, 2)
        logs = np.log(ub.sum(axis=2)).sum(axis=1)  # [8]
        for b in range(NB):
            # z col it covers flat rows it*128..; flat row r = b*T + t
            zb = zacc[:, b * ntchunk : (b + 1) * ntchunk].T.reshape(-1)  # [T]
            logz = np.log(zb).sum()
            dot = (YB2[b, 1 + j101] * AB[4 + b, 101 - j101]).sum()
            dot += (YL2[b, j100] * ML[4 + b, 100 - j100]).sum()
            ll = np.log(dot) + logs[b] + logs[4 + b] - logz
            lls.append(ll)
    return -np.sum(lls) / B


def kernel(acts, targets, act_lens, label_lens):
    acts = np.asarray(acts, np.float32)
    targets = np.asarray(targets).astype(np.int64)
    act_lens = np.asarray(act_lens)
    label_lens = np.asarray(label_lens)
    assert acts.shape == (T, B, V), acts.shape
    assert targets.shape == (B, L)
    assert (act_lens == T).all() and (label_lens == L).all(), "only full lens supported"
    assert (targets[:, 1:] != targets[:, :-1]).all(), "adjacent repeats unsupported"

    nc = _get_program(T)
    in_maps = make_in_maps(acts, targets, T)
    res = run_bass_kernel_spmd(nc, in_maps, core_ids=list(range(NCORES)))
    return np.float32(finalize(res.results, T))


if __name__ == "__main__":
    rng = np.random.default_rng(0)
    acts = rng.standard_normal((T, B, V)).astype(np.float32)
    targets = rng.integers(1, V, (B, L)).astype(np.int32)
    for bb in range(B):
        while (targets[bb, 1:] == targets[bb, :-1]).any():
            targets[bb] = rng.integers(1, V, (L,)).astype(np.int32)
    act_lens = np.full(B, T, np.int32)
    label_lens = np.full(B, L, np.int32)
    out = kernel(acts, targets, act_lens, label_lens)
    print("kernel loss:", out)
    from ctc_numpy import ctc_ref_numpy

    ref = ctc_ref_numpy(acts, targets, act_lens, label_lens)
    print("ref    loss:", ref, " rel err:", abs(out - ref) / abs(ref))


# revision 17
# speedup vs baseline: 1.0986x; 1.0303x over previous
"""CTC loss (warp-ctc semantics, size_average=True) on 8 Trainium2 NeuronCores.

Strategy (data-parallel over batch, 4 samples per core):
- Z[t,b] = sum_v exp(acts[t,b,v]) streamed as 32 [128,4000] f32 tiles over
  THREE DMA queues (sync + scalar HW DGE, gpsimd SW DGE); exp + free-dim sum
  fused in one ScalarE activation (accum_out, f32). Host does log Z in f64.
- The alpha recursion runs in the LINEAR domain, bf16, with STATES ON THE
  FREE AXIS (partition shifts are illegal on compute engines; free-dim
  offsets are free). 8 partition rows = 4 samples x {fw, bw}.
- Forward and backward DPs run SIMULTANEOUSLY in the same instructions
  (backward CTC == forward CTC on time-reversed, state-flipped data), meeting
  in the middle: 255 fused steps instead of 511. State = (y_b, ml) where
  y_b = pre-emission blank sums and ml = label alphas; per step exactly 3
  serial DVE ops (the blank emission is a per-row scalar, so it fuses into a
  scalar_tensor_tensor multiply-add):
    STT: y_b' = y_b * pb + shift(ml)
    ADD: y_l  = y_b' + ml
    MUL: ml'  = y_l * pl
  No TensorE, no cross-engine syncs on the serial chain.
- Rescale every RSC steps: accum_out on STT/MUL gives state sums free;
  reciprocal applied via tensor_scalar. Factors folded back in log on host.
- Emission tables (host pre-exp'd: pgl bf16 [8,256*100], pgb f32 [8,256]) and
  init states load over the idle gpsimd queue; result DMAs also go via
  gpsimd so they never block the streaming queues.
- Final: ll_b = log(sum_s y_fw[s]*g_bw[s]) + sum log u - sum log Z (host f64);
  loss = -mean(ll).
"""

import sys
import types

import numpy as np
import ml_dtypes

# ---- shim: provide antenv.axon_hooks (missing in this image) ----------------
_HOOK = [None]
try:
    import antenv.axon_hooks  # noqa: F401
except ImportError:
    try:
        from trn_agent_boot.trn_boot import _ntff_profile_via_ctypes

        _HOOK[0] = _ntff_profile_via_ctypes("/opt/axon/libaxon_pjrt.so")
    except Exception:
        pass
    _m = types.ModuleType("antenv.axon_hooks")
    _m.get_axon_ntff_profile_hook = lambda: _HOOK[0]
    _m.set_axon_ntff_profile_hook = lambda h: _HOOK.__setitem__(0, h)
    sys.modules["antenv.axon_hooks"] = _m
# -----------------------------------------------------------------------------

import concourse.bass as bass
import concourse.mybir as mybir
import concourse.tile as tile
from concourse.bass_utils import run_bass_kernel_spmd
from concourse.vector_clock import ScopedClock


# ---- walrus-compat patches: this walrus rejects Drains with >1 sem wait -----
def _my_drain_and_barrier(self, tick_clock, wait_clock):
    nc = self.nc
    dummy = nc.sync.nop(nofuse=True)
    wait_clock.add_sem_waits(dummy.ins, ScopedClock({None: tick_clock.global_clock}))
    si = dummy.ins.sync_info
    waits = list(si.on_wait) if si is not None else []
    if si is not None and len(waits) > 1:
        dummy.ins.sync_info = mybir.SyncInfo(
            on_wait=[waits[0]], on_update=list(si.on_update)
        )
        for w in waits[1:]:
            n = nc.sync.nop(nofuse=True)
            n.ins.sync_info = mybir.SyncInfo(on_wait=[w], on_update=[])
    nc.sync.drain()
    nc.all_engine_barrier()
    assert self.sems is not None
    popped = nc._tile_sem_poison_stack.pop()
    assert popped is self._sem_poison
    nc.clear_and_free_semaphores(list(self.sems.allocated().values()))
    nc.all_engine_barrier()


def _my_multi_engine_barrier(self, engines):
    for e in engines:
        self.engines[e].drain()
    for inst in self._sem_only_all_engine_barrier_insts(f"aeb{self.next_id()}"):
        self.engines[inst.engine].add_instruction(inst)


tile.TileContext._drain_and_barrier = _my_drain_and_barrier
bass.Bass.multi_engine_barrier = _my_multi_engine_barrier


def _split_multiwait(nc):
    """This walrus build encodes at most one sync-wait per instruction; hoist
    extra waits onto preceding nofuse NOPs on the same engine."""
    n_new = 0
    for fn in nc.m.functions:
        for blk in fn.blocks:
            insts = blk.instructions
            i = 0
            while i < len(insts):
                ins = insts[i]
                si = getattr(ins, "sync_info", None)
                if si is not None and si.on_wait and len(si.on_wait) > 1:
                    waits = list(si.on_wait)
                    ins.sync_info = mybir.SyncInfo(
                        on_wait=[waits[-1]], on_update=list(si.on_update)
                    )
                    new_nops = []
                    for w in waits[:-1]:
                        nop = mybir.InstNoOp(
                            name=f"{ins.name}_wsplit{n_new}",
                            engine=ins.engine,
                            sync_info=mybir.SyncInfo(on_wait=[w], on_update=[]),
                            bass_nofuse=True,
                        )
                        n_new += 1
                        new_nops.append(nop)
                    insts[i:i] = new_nops
                    i += len(new_nops)
                i += 1
    return nc
# -----------------------------------------------------------------------------

T, B, V, L = 512, 32, 8000, 100
NCORES = 8
NB = B // NCORES  # 4 samples per core
KS = 255  # fused fw+bw steps
NK = 256  # table slots
RSC = 32  # rescale every RSC steps
NSITES = len(range(RSC, KS, RSC))  # 7
NT2 = NB * (T // 128)  # 16 streaming tiles per core (32KB descriptors)
SBUFS = 3  # streaming tile pool depth
F32 = mybir.dt.float32
BF16 = mybir.dt.bfloat16
BFNP = ml_dtypes.bfloat16
ADD = mybir.AluOpType.add
MULT = mybir.AluOpType.mult


def build_program(t_steps=T):
    """Build the per-core Bass program (identical for all cores)."""
    assert t_steps == T
    nc = bass.Bass("TRN2", target_bir_lowering=False, debug=False)

    acts_d = nc.dram_tensor("acts", [NB * T, V], F32, kind="ExternalInput")
    pgl_d = nc.dram_tensor("pgl", [8, NK * L], BF16, kind="ExternalInput")
    pgb_d = nc.dram_tensor("pgb", [8, NK], F32, kind="ExternalInput")
    ab0_d = nc.dram_tensor("ab0", [8, 102], BF16, kind="ExternalInput")
    ml0_d = nc.dram_tensor("ml0", [8, 101], BF16, kind="ExternalInput")

    zacc_d = nc.dram_tensor("zacc", [128, NT2], F32, kind="ExternalOutput")
    ab_d = nc.dram_tensor("ab", [8, 102], BF16, kind="ExternalOutput")
    ml_d = nc.dram_tensor("ml", [8, 101], BF16, kind="ExternalOutput")
    yb2_d = nc.dram_tensor("yb2", [8, 102], BF16, kind="ExternalOutput")
    yl2_d = nc.dram_tensor("yl2", [8, 100], BF16, kind="ExternalOutput")
    ubuf_d = nc.dram_tensor("ubuf", [8, 2 * NSITES], F32, kind="ExternalOutput")

    with tile.TileContext(nc) as tc:
        with (
            tc.tile_pool(name="singles", bufs=1) as singles,
            tc.tile_pool(name="stream", bufs=SBUFS) as stream_pool,
            tc.tile_pool(name="escratch", bufs=1) as escratch_pool,
        ):
            # ---- tables + init state over the sync queue (fast, first) ------
            pgl = singles.tile([8, NK * L], BF16)
            pgb = singles.tile([8, NK], F32)
            AB = singles.tile([8, 102], BF16)  # alpha_blank, f=1..101
            ML = singles.tile([8, 101], BF16)  # f0 guard, alpha_label f=1..100
            YB = singles.tile([8, 102], BF16)  # y_blank
            YL = singles.tile([8, 100], BF16)  # y_label
            rt = singles.tile([8, 2], F32)
            ub = singles.tile([8, 2 * NSITES], F32)
            nc.scalar.dma_start(out=pgb, in_=pgb_d[:, :])
            nc.scalar.dma_start(out=AB, in_=ab0_d[:, :])
            nc.scalar.dma_start(out=ML, in_=ml0_d[:, :])
            nc.scalar.dma_start(out=pgl, in_=pgl_d[:, :])

            site = 0
            for k in range(1, KS + 1):
                at_site = k % RSC == 0 and k < KS
                # ADD1: y_b[j] = a_b[j] + a_l[j-1]
                nc.vector.tensor_add(YB[:, 1:102], AB[:, 1:102], ML[:, 0:101])
                # ADD2: y_l[j] = y_b[j] + a_l[j]
                nc.vector.tensor_add(YL[:, 0:100], YB[:, 1:101], ML[:, 1:101])
                if at_site:
                    nc.vector.tensor_scalar(
                        AB[:, 1:102], YB[:, 1:102], pgb[:, k : k + 1], 0.0,
                        op0=MULT, op1=ADD,
                        accum_out=ub[:, 2 * site : 2 * site + 1],
                    )
                    nc.vector.scalar_tensor_tensor(
                        ML[:, 1:101], YL[:, 0:100], 0.0,
                        pgl[:, k * L : k * L + 100], op0=ADD, op1=MULT,
                        accum_out=ub[:, 2 * site + 1 : 2 * site + 2],
                    )
                    nc.vector.tensor_add(
                        rt[:, 1:2],
                        ub[:, 2 * site : 2 * site + 1],
                        ub[:, 2 * site + 1 : 2 * site + 2],
                    )
                    nc.vector.reciprocal(rt[:, 0:1], rt[:, 1:2])
                    nc.vector.tensor_scalar_mul(AB[:, :], AB[:, :], rt[:, 0:1])
                    nc.vector.tensor_scalar_mul(ML[:, :], ML[:, :], rt[:, 0:1])
                    site += 1
                else:
                    nc.vector.tensor_scalar_mul(
                        AB[:, 1:102], YB[:, 1:102], pgb[:, k : k + 1]
                    )
                    nc.vector.tensor_mul(
                        ML[:, 1:101], YL[:, 0:100], pgl[:, k * L : k * L + 100]
                    )
            assert site == NSITES

            # final half-step: y_256 for the host dot product
            nc.vector.tensor_add(YB[:, 1:102], AB[:, 1:102], ML[:, 0:101])
            nc.vector.tensor_add(YL[:, 0:100], YB[:, 1:101], ML[:, 1:101])

            # ---- streaming Z = sum_v exp(acts), three DMA queues ------------
            ztile = singles.tile([128, NT2], F32)
            engs = [nc.sync, nc.scalar, nc.gpsimd]

            def tile_ap(it):
                return acts_d[it * 128 : (it + 1) * 128, :]

            tiles = {}
            for it in range(min(SBUFS, NT2)):
                ta = stream_pool.tile([128, V], F32, tag="acts")
                engs[it % 3].dma_start(out=ta, in_=tile_ap(it))
                tiles[it] = ta
            for it in range(NT2):
                ta = tiles.pop(it)
                e_t = escratch_pool.tile([128, V], BF16, tag="escr")
                nc.scalar.activation(
                    e_t, ta, mybir.ActivationFunctionType.Exp,
                    accum_out=ztile[:, it : it + 1],
                )
                nxt = it + SBUFS
                if nxt < NT2:
                    tb = stream_pool.tile([128, V], F32, tag="acts")
                    engs[nxt % 3].dma_start(out=tb, in_=tile_ap(nxt))
                    tiles[nxt] = tb

            # result DMAs emitted last so they can never sit ahead of
            # streaming work in any queue
            nc.sync.dma_start(out=ab_d[:, :], in_=AB)
            nc.sync.dma_start(out=ml_d[:, :], in_=ML)
            nc.sync.dma_start(out=yb2_d[:, :], in_=YB)
            nc.sync.dma_start(out=yl2_d[:, :], in_=YL)
            nc.gpsimd.dma_start(out=ubuf_d[:, :], in_=ub)
            nc.gpsimd.dma_start(out=zacc_d[:, :], in_=ztile)
    _split_multiwait(nc)
    return nc


_NC_CACHE = {}


def _get_program(t_steps=T):
    if t_steps not in _NC_CACHE:
        _NC_CACHE[t_steps] = build_program(t_steps)
    return _NC_CACHE[t_steps]


def make_in_maps(acts, targets, t_steps=T):
    assert t_steps == T
    in_maps = []
    karr = np.arange(NK)
    bidx = np.arange(NB)
    for c in range(NCORES):
        bs = slice(c * NB, (c + 1) * NB)
        acts_c = np.ascontiguousarray(
            acts[:, bs, :].transpose(1, 0, 2).reshape(NB * T, V)
        )
        tg = np.asarray(targets[bs], np.int64)  # [NB, L]
        a = acts[:, bs, :]  # [T, NB, V] f32

        pgb = np.empty((8, NK), np.float32)
        pgb[0:4] = np.exp(a[karr, :, 0]).T
        pgb[4:8] = np.exp(a[T - 1 - karr, :, 0]).T
        lab_fw = a[karr[:, None, None], bidx[None, :, None], tg[None, :, :]]
        lab_bw = a[
            (T - 1 - karr)[:, None, None], bidx[None, :, None], tg[None, :, ::-1]
        ]
        pgl = np.empty((8, NK, L), np.float32)
        pgl[0:4] = np.exp(lab_fw).transpose(1, 0, 2)
        pgl[4:8] = np.exp(lab_bw).transpose(1, 0, 2)
        ab0 = np.zeros((8, 102), np.float32)
        ml0 = np.zeros((8, 101), np.float32)
        for b in range(NB):
            ab0[b, 1] = np.exp(a[0, b, 0])
            ab0[4 + b, 1] = np.exp(a[T - 1, b, 0])
            ml0[b, 1] = np.exp(a[0, b, tg[b, 0]])
            ml0[4 + b, 1] = np.exp(a[T - 1, b, tg[b, L - 1]])
        in_maps.append(
            {
                "acts": acts_c,
                "pgl": np.ascontiguousarray(pgl.reshape(8, NK * L).astype(BFNP)),
                "pgb": pgb,
                "ab0": ab0.astype(BFNP),
                "ml0": ml0.astype(BFNP),
            }
        )
    return in_maps


def finalize(results, t_steps=T):
    """Host-side combine: per-sample log-likelihoods -> scalar loss (f64)."""
    assert t_steps == T
    ntchunk = T // 128
    j101 = np.arange(101)
    j100 = np.arange(100)
    lls = []
    for c in range(NCORES):
        out = results[c]
        zacc = np.asarray(out["zacc"], np.float64)  # [128, NT2]
        AB = np.asarray(out["ab"], np.float64)
        ML = np.asarray(out["ml"], np.float64)
        YB2 = np.asarray(out["yb2"], np.float64)
        YL2 = np.asarray(out["yl2"], np.float64)
        ub = np.asarray(out["ubuf"], np.float64).reshape(8, NSITES, 2)
        logs = np.log(ub.sum(axis=2)).sum(axis=1)  # [8]
        for b in range(NB):
            # z col it covers flat rows it*128..; flat row r = b*T + t
            zb = zacc[:, b * ntchunk : (b + 1) * ntchunk].T.reshape(-1)  # [T]
            logz = np.log(zb).sum()
            dot = (YB2[b, 1 + j101] * AB[4 + b, 101 - j101]).sum()
            dot += (YL2[b, j100] * ML[4 + b, 100 - j100]).sum()
            ll = np.log(dot) + logs[b] + logs[4 + b] - logz
            lls.append(ll)
    return -np.sum(lls) / B


def kernel(acts, targets, act_lens, label_lens):
    acts = np.asarray(acts, np.float32)
    targets = np.asarray(targets).astype(np.int64)
    act_lens = np.asarray(act_lens)
    label_lens = np.asarray(label_lens)
    assert acts.shape == (T, B, V), acts.shape
    assert targets.shape == (B, L)
    assert (act_lens == T).all() and (label_lens == L).all(), "only full lens supported"
    assert (targets[:, 1:] != targets[:, :-1]).all(), "adjacent repeats unsupported"

    nc = _get_program(T)
    in_maps = make_in_maps(acts, targets, T)
    res = run_bass_kernel_spmd(nc, in_maps, core_ids=list(range(NCORES)))
    return np.float32(finalize(res.results, T))


if __name__ == "__main__":
    rng = np.random.default_rng(0)
    acts = rng.standard_normal((T, B, V)).astype(np.float32)
    targets = rng.integers(1, V, (B, L)).astype(np.int32)
    for bb in range(B):
        while (targets[bb, 1:] == targets[bb, :-1]).any():
            targets[bb] = rng.integers(1, V, (L,)).astype(np.int32)
    act_lens = np.full(B, T, np.int32)
    label_lens = np.full(B, L, np.int32)
    out = kernel(acts, targets, act_lens, label_lens)
    print("kernel loss:", out)
    from ctc_numpy import ctc_ref_numpy

    ref = ctc_ref_numpy(acts, targets, act_lens, label_lens)
    print("ref    loss:", ref, " rel err:", abs(out - ref) / abs(ref))
